# revision 18
# baseline (speedup 1.0000x reference)
"""Trainium2 Bass kernel for nn_Attention (dense transformer block).

Computation (per batch b of 16):
  qkv = BN(qkv_w @ x)            # 1x1 conv + BN, x: [256, 1024]
  per head h (8): q,k: [16,1024], v: [32,1024]
  attn = softmax(q^T k * scale); out = v @ attn^T
  pe   = BN(dwconv3x3(v_img))
  y    = BN(proj_w @ (out + pe))

Sharding: data-parallel over batch. 16 batches / 8 cores = 2 per core.
All BN folds / weight transposes / layout packing happen on host (numpy).

Device-side layout tricks:
 - q/k packed at 32-aligned partition strips so the K=16 logits matmuls
   row-tile into concurrent PE strips.
 - S_T = k^T q computed *transposed* ([j,i]) so softmax's contraction dim
   lands on partitions for the AV matmul -- no transposes needed.
 - v^T computed directly from x via x^T @ Wv with an appended ones column:
   the AV matmul then yields the softmax denominator for free (M=33).
 - exp on ACT reads [128,2048] PSUM tiles (2 heads per tile) to amortize
   per-instruction overhead. ACT is the bottleneck engine (~8.4M exp/batch).
 - softmax normalization: row-sums DMA'd to a [8,1024] tile, reciprocal via
   1 custom-DVE op, broadcast back with a K=8 selector matmul.
 - depthwise conv: 9 scalar_tensor_tensor DVE ops (shifted APs, zero pad
   via clipped slices) accumulating into the attention output.
"""

import numpy as np

B, C, HH, WW = 16, 256, 32, 32
N = HH * WW  # 1024
NH, HD, KD = 8, 32, 16
SCALE = KD ** -0.5
QKV_CH = C + 2 * KD * NH  # 512
EPS = 1e-5
NCORES = 8
BPC = B // NCORES  # batches per core

# matmul dtype: float32r = fp32 bits, PE reduced-precision full-rate mode.
USE_F32R = True

_CACHE = {}


def _build_program():
    import concourse.bass as bass
    import concourse.bacc as bacc
    import concourse.tile as tile
    from concourse import mybir

    f32 = mybir.dt.float32
    f32r = mybir.dt.float32r
    AF = mybir.ActivationFunctionType
    ALU = mybir.AluOpType

    mdt = f32r if USE_F32R else f32
    bdt = mybir.dt.bfloat16

    def fr(ap):
        return ap

    nc = bacc.Bacc("TRN2", target_bir_lowering=False, debug=False)

    # ---- DRAM I/O ----
    x_d = nc.dram_tensor("x", [BPC, C, N], mdt, kind="ExternalInput").ap()
    wqk_d = nc.dram_tensor("wqk", [2, 128, 512], mdt, kind="ExternalInput").ap()
    bqk_d = nc.dram_tensor("bqk", [4, 128, 1], f32, kind="ExternalInput").ap()
    wv_d = nc.dram_tensor("wv", [2, 128, 264], mdt, kind="ExternalInput").ap()
    wvl_d = nc.dram_tensor("wvl", [1, 264], mdt, kind="ExternalInput").ap()
    wvcm_d = nc.dram_tensor("wvcm", [2, 128, 256], mdt, kind="ExternalInput").ap()
    bvcm_d = nc.dram_tensor("bvcm", [2, 128, 1], f32, kind="ExternalInput").ap()
    w9_d = nc.dram_tensor("w9", [2, 128, 9], f32, kind="ExternalInput").ap()
    bpe_d = nc.dram_tensor("bpe", [2, 128, 1], f32, kind="ExternalInput").ap()
    wpj_d = nc.dram_tensor("wpj", [2, 128, 256], f32, kind="ExternalInput").ap()
    bpj_d = nc.dram_tensor("bpj", [2, 128, 1], f32, kind="ExternalInput").ap()
    sel_d = nc.dram_tensor("sel", [2, 8, 128], f32, kind="ExternalInput").ap()
    ones_d = nc.dram_tensor("onesr", [1, 128], mdt, kind="ExternalInput").ap()
    y_d = nc.dram_tensor("y", [BPC, C, N], f32, kind="ExternalOutput").ap()

    with tile.TileContext(nc) as tc:
        with (
            tc.tile_pool(name="consts", bufs=1) as consts,
            tc.tile_pool(name="xin", bufs=2) as xin,
            tc.tile_pool(name="qk", bufs=2) as qkp,
            tc.tile_pool(name="vt", bufs=2) as vtp,
            tc.tile_pool(name="vcm", bufs=2) as vcmp,
            tc.tile_pool(name="et", bufs=32) as etp,
            tc.tile_pool(name="ztmp", bufs=2) as ztmpp,
            tc.tile_pool(name="zz", bufs=2) as zp,
            tc.tile_pool(name="rr", bufs=2) as rp,
            tc.tile_pool(name="yy", bufs=2) as yp,
            tc.tile_pool(name="ps_s", bufs=2, space="PSUM") as ps_s,
            tc.tile_pool(name="ps_o", bufs=1, space="PSUM") as ps_o,
            tc.tile_pool(name="ps_sm", bufs=1, space="PSUM") as ps_sm,
        ):
            # ---- load constants ----
            wqk_sb = []
            for kt in range(2):
                t = consts.tile([128, 512], mdt, tag=f"wqk{kt}", name=f"wqk{kt}")
                nc.sync.dma_start(out=t, in_=wqk_d[kt])
                wqk_sb.append(t)
            bqk_sb = []
            for mt in range(4):
                t = consts.tile([128, 1], f32, tag=f"bqk{mt}", name=f"bqk{mt}")
                nc.sync.dma_start(out=t, in_=bqk_d[mt])
                bqk_sb.append(t)
            wv_sb = []
            for kt in range(2):
                t = consts.tile([128, 264], mdt, tag=f"wv{kt}", name=f"wv{kt}")
                nc.sync.dma_start(out=t, in_=wv_d[kt])
                wv_sb.append(t)
            wvl_sb = consts.tile([1, 264], mdt, tag="wvl", name="wvl")
            nc.sync.dma_start(out=wvl_sb, in_=wvl_d)
            wvcm_sb = []
            bvcm_sb = []
            w9_sb = []
            bpe_sb = []
            wpj_sb = []
            bpj_sb = []
            for kt in range(2):
                t = consts.tile([128, 256], mdt, tag=f"wvcm{kt}", name=f"wvcm{kt}")
                nc.sync.dma_start(out=t, in_=wvcm_d[kt])
                wvcm_sb.append(t)
                t = consts.tile([128, 1], f32, tag=f"bvcm{kt}", name=f"bvcm{kt}")
                nc.sync.dma_start(out=t, in_=bvcm_d[kt])
                bvcm_sb.append(t)
                t = consts.tile([128, 9], f32, tag=f"w9{kt}", name=f"w9{kt}")
                nc.sync.dma_start(out=t, in_=w9_d[kt])
                w9_sb.append(t)
                t = consts.tile([128, 1], f32, tag=f"bpe{kt}", name=f"bpe{kt}")
                nc.sync.dma_start(out=t, in_=bpe_d[kt])
                bpe_sb.append(t)
                t = consts.tile([128, 256], f32, tag=f"wpj{kt}", name=f"wpj{kt}")
                nc.sync.dma_start(out=t, in_=wpj_d[kt])
                wpj_sb.append(t)
                t = consts.tile([128, 1], f32, tag=f"bpj{kt}", name=f"bpj{kt}")
                nc.sync.dma_start(out=t, in_=bpj_d[kt])
                bpj_sb.append(t)
            sel_sb = []
            for tt in range(2):
                t = consts.tile([8, 128], f32, tag=f"sel{tt}", name=f"sel{tt}")
                nc.sync.dma_start(out=t, in_=sel_d[tt])
                sel_sb.append(t)
            ones_sb = consts.tile([1, 128], mdt, tag="ones", name="ones")
            nc.sync.dma_start(out=ones_sb, in_=ones_d)

            for b in range(BPC):
                # ---- load x ----
                x_sb = []
                for kt in range(2):
                    t = xin.tile([128, N], mdt, tag=f"x{kt}", name=f"x{kt}")
                    nc.sync.dma_start(out=t, in_=x_d[b, kt * 128:(kt + 1) * 128, :])
                    x_sb.append(t)

                # ---- stage 1: q/k rows (padded strip layout) ----
                # m-tile 0: QA (q of heads 0-3), 1: KA, 2: QB (heads 4-7), 3: KB
                qk_sb = []
                for mt in range(4):
                    ps = ps_sm.tile([128, N], f32, tag="mm", name="psmm")
                    for kt in range(2):
                        for ns in range(2):
                            nc.tensor.matmul(
                                ps[:, ns * 512:(ns + 1) * 512],
                                fr(wqk_sb[kt][:, mt * 128:(mt + 1) * 128]),
                                fr(x_sb[kt][:, ns * 512:(ns + 1) * 512]),
                                start=(kt == 0), stop=(kt == 1),
                            )
                    t = qkp.tile([128, N], bdt, tag=f"qk{mt}", name=f"qk{mt}")
                    nc.vector.tensor_scalar_add(t, ps, bqk_sb[mt])
                    qk_sb.append(t)

                # ---- stage 1b: vT_aug = xa^T @ Wv  ([1024, 264]) ----
                vt_sb = []
                for nt in range(8):
                    ps = ps_sm.tile([128, 264], f32, tag="mm", name="psvt")
                    nc.tensor.matmul(
                        ps, fr(x_sb[0][:, nt * 128:(nt + 1) * 128]), fr(wv_sb[0]),
                        start=True, stop=False)
                    nc.tensor.matmul(
                        ps, fr(x_sb[1][:, nt * 128:(nt + 1) * 128]), fr(wv_sb[1]),
                        start=False, stop=False)
                    nc.tensor.matmul(
                        ps, fr(ones_sb), fr(wvl_sb),
                        start=False, stop=True)
                    t = vtp.tile([128, 264], bdt, tag=f"vt{nt}", name=f"vt{nt}")
                    nc.vector.tensor_copy(t, ps)
                    vt_sb.append(t)

                # ---- stage 1c: v channel-major [256, 1024] ----
                vcm_sb = []
                for mt in range(2):
                    ps = ps_sm.tile([128, N], f32, tag="mm", name="psmm")
                    for kt in range(2):
                        for ns in range(2):
                            nc.tensor.matmul(
                                ps[:, ns * 512:(ns + 1) * 512],
                                fr(wvcm_sb[kt][:, mt * 128:(mt + 1) * 128]),
                                fr(x_sb[kt][:, ns * 512:(ns + 1) * 512]),
                                start=(kt == 0), stop=(kt == 1),
                            )
                    t = vcmp.tile([128, N], f32, tag=f"vcm{mt}", name=f"vcm{mt}")
                    nc.vector.tensor_scalar_add(t, ps, bvcm_sb[mt])
                    vcm_sb.append(t)

                # ---- attention: head pairs; strip-alternating S_T; deferred AV ----
                z_sb = [zp.tile([128, N], f32, tag=f"z{t}", name=f"z{t}") for t in range(2)]
                r_sb = rp.tile([8, N], f32, tag="r", name="rsum")
                # software pipeline over pairs: pair p's S_T/exp (phase A)
                # interleaved in program order with pair p-1's AV matmuls so
                # full-row K=128 AV work alternates with K=16 S_T work.
                prev = None  # (heads, ets) of previous pair

                def av_head(h, ets_h, p_done):
                    pso = ps_o.tile([33, N], f32, tag="av", name="psav")
                    for jt in range(8):
                        for ns in range(2):
                            nc.tensor.matmul(
                                pso[:, ns * 512:(ns + 1) * 512],
                                fr(vt_sb[jt][:, 33 * h: 33 * h + 33]),
                                fr(ets_h[jt][:, ns * 512:(ns + 1) * 512]),
                                start=(jt == 0), stop=(jt == 7),
                            )
                    zt = ztmpp.tile([33, N], f32, tag="ztmp", name="ztmp")
                    nc.vector.tensor_copy(zt, pso)
                    nc.sync.dma_start(out=r_sb[h: h + 1, :], in_=zt[32:33, :])
                    nc.sync.dma_start(
                        out=z_sb[h // 4][32 * (h % 4): 32 * (h % 4) + 32, :],
                        in_=zt[0:32, :],
                    )

                def av_slot(prev_state, jt):
                    # 4 AV matmuls of the previous pair, spread across slots
                    (pheads, pets, psos) = prev_state
                    hi = 0 if jt < 4 else 1
                    if psos[hi] is None:
                        psos[hi] = ps_o.tile([33, N], f32, tag="av", name="psav")
                    pso = psos[hi]
                    for avjt in (2 * (jt % 4), 2 * (jt % 4) + 1):
                        for ns in range(2):
                            nc.tensor.matmul(
                                pso[:, ns * 512:(ns + 1) * 512],
                                fr(vt_sb[avjt][:, 33 * pheads[hi]: 33 * pheads[hi] + 33]),
                                fr(pets[hi][avjt][:, ns * 512:(ns + 1) * 512]),
                                start=(avjt == 0), stop=(avjt == 7),
                            )
                    if jt == 3 or jt == 7:
                        # head done: evict + row-sum + compact
                        h = pheads[hi]
                        zt = ztmpp.tile([33, N], f32, tag="ztmp", name="ztmp")
                        nc.vector.tensor_copy(zt, pso)
                        nc.sync.dma_start(out=r_sb[h: h + 1, :], in_=zt[32:33, :])
                        nc.sync.dma_start(
                            out=z_sb[h // 4][32 * (h % 4): 32 * (h % 4) + 32, :],
                            in_=zt[0:32, :],
                        )

                for p in range(4):
                    heads = (2 * p, 2 * p + 1)
                    strips = [32 * (h % 4) for h in heads]
                    qts = [qk_sb[0 if h < 4 else 2] for h in heads]
                    kts = [qk_sb[1 if h < 4 else 3] for h in heads]
                    ets = [[None] * 8, [None] * 8]
                    for jt in range(8):
                        psss = []
                        for hi in range(2):
                            pss = ps_s.tile([128, N], f32, tag="st", name="psst")
                            psss.append(pss)
                        for ns in range(2):
                            for hi in range(2):
                                nc.tensor.matmul(
                                    psss[hi][:, ns * 512:(ns + 1) * 512],
                                    fr(kts[hi][strips[hi]:strips[hi] + 16, jt * 128:(jt + 1) * 128]),
                                    fr(qts[hi][strips[hi]:strips[hi] + 16, ns * 512:(ns + 1) * 512]),
                                    start=True, stop=True,
                                    tile_position=(strips[hi], 0),
                                )
                        if prev is not None:
                            av_slot(prev, jt)
                        for hi in range(2):
                            et = etp.tile([128, N], bdt, tag="et", name="et")
                            nc.scalar.activation(et, psss[hi], AF.Exp, scale=SCALE)
                            ets[hi][jt] = et
                    prev = (heads, ets, [None, None])

                # drain: AV of the last pair
                for hi in range(2):
                    av_head(prev[0][hi], prev[1][hi], None)

                # ---- softmax normalization ----
                rinv_sb = rp.tile([8, N], f32, tag="rinv", name="rinv")
                nc.vector.reciprocal_approx_fast(out=rinv_sb, in_=r_sb)
                for tt in range(2):
                    ps = ps_sm.tile([128, N], f32, tag="mm", name="psmm")
                    for ns in range(2):
                        nc.tensor.matmul(
                            ps[:, ns * 512:(ns + 1) * 512],
                            fr(sel_sb[tt]),
                            fr(rinv_sb[:, ns * 512:(ns + 1) * 512]),
                            start=True, stop=True,
                        )
                    nc.vector.tensor_tensor(z_sb[tt], z_sb[tt], ps, ALU.mult)

                # ---- depthwise 3x3 conv accumulated into z ----
                for tt in range(2):
                    zv = z_sb[tt].rearrange("p (h w) -> p h w", h=HH)
                    vv = vcm_sb[tt].rearrange("p (h w) -> p h w", h=HH)
                    # center tap on the flat view: adds BN bias once
                    nc.vector.affine_then_add(
                        out=z_sb[tt], in0=vcm_sb[tt], in1=z_sb[tt],
                        scale=w9_sb[tt][:, 4:5], bias=bpe_sb[tt],
                    )
                    for tap in range(9):
                        if tap == 4:
                            continue
                        dy, dx = tap // 3 - 1, tap % 3 - 1
                        oh0, oh1 = max(0, -dy), HH - max(0, dy)
                        ow0, ow1 = max(0, -dx), WW - max(0, dx)
                        nc.vector.scalar_tensor_tensor(
                            out=zv[:, oh0:oh1, ow0:ow1],
                            in0=vv[:, oh0 + dy:oh1 + dy, ow0 + dx:ow1 + dx],
                            scalar=w9_sb[tt][:, tap:tap + 1],
                            in1=zv[:, oh0:oh1, ow0:ow1],
                            op0=ALU.mult, op1=ALU.add,
                        )

                # ---- proj + BN + store ----
                for mt in range(2):
                    ps = ps_sm.tile([128, N], f32, tag="mm", name="psmm")
                    for kt in range(2):
                        for ns in range(2):
                            nc.tensor.matmul(
                                ps[:, ns * 512:(ns + 1) * 512],
                                fr(wpj_sb[kt][:, mt * 128:(mt + 1) * 128]),
                                fr(z_sb[kt][:, ns * 512:(ns + 1) * 512]),
                                start=(kt == 0), stop=(kt == 1),
                            )
                    yt = yp.tile([128, N], f32, tag=f"y{mt}", name=f"y{mt}")
                    nc.vector.tensor_scalar_add(yt, ps, bpj_sb[mt])
                    nc.sync.dma_start(out=y_d[b, mt * 128:(mt + 1) * 128, :], in_=yt)

    nc.compile()
    return nc


def _prep_weights(qkv_w, qkv_g, qkv_b, qkv_m, qkv_v,
                  pe_w, pe_g, pe_b, pe_m, pe_v,
                  proj_w, proj_g, proj_b, proj_m, proj_v):
    f = np.float32
    s1 = (qkv_g / np.sqrt(qkv_v + EPS)).astype(f)
    t1 = (qkv_b - qkv_m * s1).astype(f)
    w1 = (qkv_w * s1[:, None]).astype(f)  # [512, 256]

    # q/k padded strip layout: 4 m-tiles of 128 cols.
    wqk = np.zeros((C, 512), f)
    bqk = np.zeros((4, 128, 1), f)
    for h in range(NH):
        g, m = divmod(h, 4)  # g: 0 = heads 0-3, 1 = heads 4-7
        qrows = slice(64 * h, 64 * h + 16)
        krows = slice(64 * h + 16, 64 * h + 32)
        wqk[:, 256 * g + 32 * m: 256 * g + 32 * m + 16] = w1[qrows].T
        wqk[:, 256 * g + 128 + 32 * m: 256 * g + 128 + 32 * m + 16] = w1[krows].T
        bqk[2 * g, 32 * m: 32 * m + 16, 0] = t1[qrows]
        bqk[2 * g + 1, 32 * m: 32 * m + 16, 0] = t1[krows]

    # vT_aug weights [257, 264]
    wva = np.zeros((C + 1, 264), f)
    for h in range(NH):
        vrows = slice(64 * h + 32, 64 * h + 64)
        wva[:C, 33 * h: 33 * h + 32] = w1[vrows].T
        wva[C, 33 * h: 33 * h + 32] = t1[vrows]
        wva[C, 33 * h + 32] = 1.0
    wv = wva[:C].reshape(2, 128, 264)
    wvl = wva[C:C + 1]  # [1, 264]

    # v channel-major weights (canonical order c = 32h + d)
    wvcm = np.zeros((C, C), f)
    bvcm = np.zeros((C,), f)
    for h in range(NH):
        vrows = slice(64 * h + 32, 64 * h + 64)
        wvcm[:, 32 * h: 32 * h + 32] = w1[vrows].T
        bvcm[32 * h: 32 * h + 32] = t1[vrows]
    wvcm = wvcm.reshape(2, 128, 256)
    bvcm = bvcm.reshape(2, 128, 1)

    # depthwise conv, pe BN folded
    s2 = (pe_g / np.sqrt(pe_v + EPS)).astype(f)
    t2 = (pe_b - pe_m * s2).astype(f)
    w9 = (pe_w[:, 0].reshape(C, 9) * s2[:, None]).astype(f).reshape(2, 128, 9)
    bpe = t2.reshape(2, 128, 1)

    # proj, BN folded, transposed
    s3 = (proj_g / np.sqrt(proj_v + EPS)).astype(f)
    t3 = (proj_b - proj_m * s3).astype(f)
    w3 = (proj_w * s3[:, None]).astype(f)
    wpj = np.ascontiguousarray(w3.T).reshape(2, 128, 256)
    bpj = t3.reshape(2, 128, 1)

    # selector for rinv broadcast: z tile t row m -> head 4t + m//32
    sel = np.zeros((2, 8, 128), f)
    for tt in range(2):
        for m in range(128):
            sel[tt, 4 * tt + m // 32, m] = 1.0

    return dict(wqk=wqk.reshape(2, 128, 512), bqk=bqk, wv=wv, wvl=wvl,
                wvcm=wvcm, bvcm=bvcm, w9=w9, bpe=bpe, wpj=wpj, bpj=bpj, sel=sel,
                onesr=np.ones((1, 128), f))


def _round_f32r(a):
    """Round fp32 to the PE's fp32r format (11 explicit mantissa bits)."""
    b = np.ascontiguousarray(a, np.float32).view(np.uint32)
    return ((b + np.uint32(0x800)) & np.uint32(0xFFFFF000)).view(np.float32)


def kernel(**inputs):
    from concourse.bass_utils import run_bass_kernel_spmd

    x = np.asarray(inputs["x"], dtype=np.float32)  # [16, 256, 32, 32]
    w = _prep_weights(
        np.asarray(inputs["qkv_w"], np.float32), np.asarray(inputs["qkv_g"], np.float32),
        np.asarray(inputs["qkv_b"], np.float32), np.asarray(inputs["qkv_m"], np.float32),
        np.asarray(inputs["qkv_v"], np.float32),
        np.asarray(inputs["pe_w"], np.float32), np.asarray(inputs["pe_g"], np.float32),
        np.asarray(inputs["pe_b"], np.float32), np.asarray(inputs["pe_m"], np.float32),
        np.asarray(inputs["pe_v"], np.float32),
        np.asarray(inputs["proj_w"], np.float32), np.asarray(inputs["proj_g"], np.float32),
        np.asarray(inputs["proj_b"], np.float32), np.asarray(inputs["proj_m"], np.float32),
        np.asarray(inputs["proj_v"], np.float32),
    )

    if "nc" not in _CACHE:
        _CACHE["nc"] = _build_program()
    nc = _CACHE["nc"]

    if USE_F32R:
        for k in ("wqk", "wv", "wvl", "wvcm"):
            w[k] = _round_f32r(w[k])
        x = _round_f32r(x)
    xr = x.reshape(B, C, N)
    in_maps = []
    for c in range(NCORES):
        m = {"x": np.ascontiguousarray(xr[c * BPC:(c + 1) * BPC])}
        m.update(w)
        in_maps.append(m)

    res = run_bass_kernel_spmd(nc, in_maps, core_ids=list(range(NCORES)))
    y = np.concatenate([res.results[c]["y"] for c in range(NCORES)], axis=0)
    return y.reshape(B, C, HH, WW).astype(np.float32)


# revision 21
# speedup vs baseline: 1.0929x; 1.0929x over previous
"""Trainium2 Bass kernel for nn_Attention (dense transformer block).

Computation (per batch b of 16):
  qkv = BN(qkv_w @ x)            # 1x1 conv + BN, x: [256, 1024]
  per head h (8): q,k: [16,1024], v: [32,1024]
  attn = softmax(q^T k * scale); out = v @ attn^T
  pe   = BN(dwconv3x3(v_img))
  y    = BN(proj_w @ (out + pe))

Sharding: data-parallel over batch. 16 batches / 8 cores = 2 per core.
All BN folds / weight transposes / layout packing happen on host (numpy).

Device-side layout tricks:
 - q/k packed at 32-aligned partition strips so the K=16 logits matmuls
   row-tile into concurrent PE strips.
 - S_T = k^T q computed *transposed* ([j,i]) so softmax's contraction dim
   lands on partitions for the AV matmul -- no transposes needed.
 - v^T computed directly from x via x^T @ Wv with an appended ones column:
   the AV matmul then yields the softmax denominator for free (M=33).
 - attention runs per head-pair: S_T matmuls for the two heads alternate
   32-row PE strips (tile_position row tiling) and their AV matmuls are
   deferred into bf16 E-tiles so they overlap the next pair's S_T/exp.
 - exp on ACT reads [128,1024] PSUM tiles; q/k/E/vT are bf16 (softmax
   normalization cancels most of the rounding), big GEMMs are fp32r.
 - softmax normalization: row-sums DMA'd to a [8,1024] tile, reciprocal via
   1 custom-DVE op, broadcast back with a K=8 selector matmul.
 - depthwise conv: 9 scalar_tensor_tensor DVE ops (shifted APs, zero pad
   via clipped slices) accumulating into the attention output.
PSUM budget (8 banks): S_T producer 2x[128,1024] (4) + AV accumulator
[33,1024] (2) + stage-1/proj/bcast pool [128,1024] (2).
"""

import numpy as np

B, C, HH, WW = 16, 256, 32, 32
N = HH * WW  # 1024
NH, HD, KD = 8, 32, 16
SCALE = KD ** -0.5
QKV_CH = C + 2 * KD * NH  # 512
EPS = 1e-5
NCORES = 8
BPC = B // NCORES  # batches per core

# matmul dtype: float32r = fp32 bits, PE reduced-precision full-rate mode.
USE_F32R = True

_CACHE = {}


def _build_program():
    import concourse.bass as bass
    import concourse.bacc as bacc
    import concourse.tile as tile
    from concourse import mybir

    f32 = mybir.dt.float32
    f32r = mybir.dt.float32r
    AF = mybir.ActivationFunctionType
    ALU = mybir.AluOpType

    mdt = f32r if USE_F32R else f32
    bdt = mybir.dt.bfloat16

    def fr(ap):
        return ap

    nc = bacc.Bacc("TRN2", target_bir_lowering=False, debug=False)

    # ---- DRAM I/O ----
    x_d = nc.dram_tensor("x", [BPC, C, N], mdt, kind="ExternalInput").ap()
    wqk_d = nc.dram_tensor("wqk", [2, 128, 512], mdt, kind="ExternalInput").ap()
    bqk_d = nc.dram_tensor("bqk", [4, 128, 1], f32, kind="ExternalInput").ap()
    wv_d = nc.dram_tensor("wv", [2, 128, 264], mdt, kind="ExternalInput").ap()
    wvl_d = nc.dram_tensor("wvl", [128, 264], f32, kind="ExternalInput").ap()
    wvcm_d = nc.dram_tensor("wvcm", [2, 128, 256], mdt, kind="ExternalInput").ap()
    bvcm_d = nc.dram_tensor("bvcm", [2, 128, 1], f32, kind="ExternalInput").ap()
    w9_d = nc.dram_tensor("w9", [2, 128, 9], f32, kind="ExternalInput").ap()
    bpe_d = nc.dram_tensor("bpe", [2, 128, 1], f32, kind="ExternalInput").ap()
    wpj_d = nc.dram_tensor("wpj", [2, 128, 256], f32, kind="ExternalInput").ap()
    bpj_d = nc.dram_tensor("bpj", [2, 128, 1], f32, kind="ExternalInput").ap()
    sel_d = nc.dram_tensor("sel", [2, 8, 128], f32, kind="ExternalInput").ap()
    y_d = nc.dram_tensor("y", [BPC, C, N], f32, kind="ExternalOutput").ap()

    with tile.TileContext(nc) as tc:
        with (
            tc.tile_pool(name="consts", bufs=1) as consts,
            tc.tile_pool(name="xin", bufs=2) as xin,
            tc.tile_pool(name="qk", bufs=2) as qkp,
            tc.tile_pool(name="vt", bufs=2) as vtp,
            tc.tile_pool(name="vcm", bufs=2) as vcmp,
            tc.tile_pool(name="et", bufs=32) as etp,
            tc.tile_pool(name="ztmp", bufs=2) as ztmpp,
            tc.tile_pool(name="zz", bufs=2) as zp,
            tc.tile_pool(name="rr", bufs=2) as rp,
            tc.tile_pool(name="yy", bufs=2) as yp,
            tc.tile_pool(name="ps_s", bufs=2, space="PSUM") as ps_s,
            tc.tile_pool(name="ps_o", bufs=1, space="PSUM") as ps_o,
            tc.tile_pool(name="ps_sm", bufs=1, space="PSUM") as ps_sm,
        ):
            # ---- load constants ----
            wqk_sb = []
            for kt in range(2):
                t = consts.tile([128, 512], mdt, tag=f"wqk{kt}", name=f"wqk{kt}")
                nc.sync.dma_start(out=t, in_=wqk_d[kt])
                wqk_sb.append(t)
            bqk_sb = []
            for mt in range(4):
                t = consts.tile([128, 1], f32, tag=f"bqk{mt}", name=f"bqk{mt}")
                nc.sync.dma_start(out=t, in_=bqk_d[mt])
                bqk_sb.append(t)
            wv_sb = []
            for kt in range(2):
                t = consts.tile([128, 264], mdt, tag=f"wv{kt}", name=f"wv{kt}")
                nc.sync.dma_start(out=t, in_=wv_d[kt])
                wv_sb.append(t)
            wvl_sb = consts.tile([128, 264], f32, tag="wvl", name="wvl")
            nc.sync.dma_start(out=wvl_sb, in_=wvl_d)
            wvcm_sb = []
            bvcm_sb = []
            w9_sb = []
            bpe_sb = []
            wpj_sb = []
            bpj_sb = []
            for kt in range(2):
                t = consts.tile([128, 256], mdt, tag=f"wvcm{kt}", name=f"wvcm{kt}")
                nc.sync.dma_start(out=t, in_=wvcm_d[kt])
                wvcm_sb.append(t)
                t = consts.tile([128, 1], f32, tag=f"bvcm{kt}", name=f"bvcm{kt}")
                nc.sync.dma_start(out=t, in_=bvcm_d[kt])
                bvcm_sb.append(t)
                t = consts.tile([128, 9], f32, tag=f"w9{kt}", name=f"w9{kt}")
                nc.sync.dma_start(out=t, in_=w9_d[kt])
                w9_sb.append(t)
                t = consts.tile([128, 1], f32, tag=f"bpe{kt}", name=f"bpe{kt}")
                nc.sync.dma_start(out=t, in_=bpe_d[kt])
                bpe_sb.append(t)
                t = consts.tile([128, 256], f32, tag=f"wpj{kt}", name=f"wpj{kt}")
                nc.sync.dma_start(out=t, in_=wpj_d[kt])
                wpj_sb.append(t)
                t = consts.tile([128, 1], f32, tag=f"bpj{kt}", name=f"bpj{kt}")
                nc.sync.dma_start(out=t, in_=bpj_d[kt])
                bpj_sb.append(t)
            sel_sb = []
            for tt in range(2):
                t = consts.tile([8, 128], f32, tag=f"sel{tt}", name=f"sel{tt}")
                nc.sync.dma_start(out=t, in_=sel_d[tt])
                sel_sb.append(t)

            for b in range(BPC):
                # ---- load x ----
                x_sb = []
                for kt in range(2):
                    t = xin.tile([128, N], mdt, tag=f"x{kt}", name=f"x{kt}")
                    nc.sync.dma_start(out=t, in_=x_d[b, kt * 128:(kt + 1) * 128, :])
                    x_sb.append(t)

                # ---- stage 1: q/k rows (padded strip layout) ----
                # m-tile 0: QA (q of heads 0-3), 1: KA, 2: QB (heads 4-7), 3: KB
                qk_sb = []
                for mt in range(4):
                    ps = ps_sm.tile([128, N], f32, tag="mm", name="psmm")
                    for kt in range(2):
                        for ns in range(2):
                            nc.tensor.matmul(
                                ps[:, ns * 512:(ns + 1) * 512],
                                fr(wqk_sb[kt][:, mt * 128:(mt + 1) * 128]),
                                fr(x_sb[kt][:, ns * 512:(ns + 1) * 512]),
                                start=(kt == 0), stop=(kt == 1),
                            )
                    t = qkp.tile([128, N], bdt, tag=f"qk{mt}", name=f"qk{mt}")
                    nc.vector.tensor_scalar_add(t, ps, bqk_sb[mt])
                    qk_sb.append(t)

                # ---- stage 1b: vT_aug = xa^T @ Wv  ([1024, 264]) ----
                vt_sb = []
                for nt in range(8):
                    ps = ps_sm.tile([128, 264], f32, tag="mm", name="psvt")
                    nc.tensor.matmul(
                        ps, fr(x_sb[0][:, nt * 128:(nt + 1) * 128]), fr(wv_sb[0]),
                        start=True, stop=False)
                    nc.tensor.matmul(
                        ps, fr(x_sb[1][:, nt * 128:(nt + 1) * 128]), fr(wv_sb[1]),
                        start=False, stop=True)
                    t = vtp.tile([128, 264], bdt, tag=f"vt{nt}", name=f"vt{nt}")
                    nc.vector.tensor_add(t, ps, wvl_sb)
                    vt_sb.append(t)

                # ---- stage 1c: v channel-major [256, 1024] ----
                vcm_sb = []
                for mt in range(2):
                    ps = ps_sm.tile([128, N], f32, tag="mm", name="psmm")
                    for kt in range(2):
                        for ns in range(2):
                            nc.tensor.matmul(
                                ps[:, ns * 512:(ns + 1) * 512],
                                fr(wvcm_sb[kt][:, mt * 128:(mt + 1) * 128]),
                                fr(x_sb[kt][:, ns * 512:(ns + 1) * 512]),
                                start=(kt == 0), stop=(kt == 1),
                            )
                    t = vcmp.tile([128, N], f32, tag=f"vcm{mt}", name=f"vcm{mt}")
                    nc.vector.tensor_scalar_add(t, ps, bvcm_sb[mt])
                    vcm_sb.append(t)

                # ---- attention: head pairs; strip-alternating S_T; deferred AV ----
                z_sb = [zp.tile([128, N], f32, tag=f"z{t}", name=f"z{t}") for t in range(2)]
                r_sb = rp.tile([8, N], f32, tag="r", name="rsum")
                for p in range(4):
                    heads = (2 * p, 2 * p + 1)
                    strips = [32 * (h % 4) for h in heads]
                    qts = [qk_sb[0 if h < 4 else 2] for h in heads]
                    kts = [qk_sb[1 if h < 4 else 3] for h in heads]
                    ets = [[None] * 8, [None] * 8]
                    # phase A: S_T + exp for both heads, jt-interleaved
                    for jt in range(8):
                        psss = []
                        for hi in range(2):
                            pss = ps_s.tile([128, N], f32, tag="st", name="psst")
                            psss.append(pss)
                        for ns in range(2):
                            for hi in range(2):
                                nc.tensor.matmul(
                                    psss[hi][:, ns * 512:(ns + 1) * 512],
                                    fr(kts[hi][strips[hi]:strips[hi] + 16, jt * 128:(jt + 1) * 128]),
                                    fr(qts[hi][strips[hi]:strips[hi] + 16, ns * 512:(ns + 1) * 512]),
                                    start=True, stop=True,
                                    tile_position=(strips[hi], 0),
                                )
                        for hi in range(2):
                            et = etp.tile([128, N], bdt, tag="et", name="et")
                            nc.scalar.activation(et, psss[hi], AF.Exp, scale=SCALE)
                            ets[hi][jt] = et
                    # phase B: AV per head (overlaps next pair's phase A on PE)
                    for hi, h in enumerate(heads):
                        pso = ps_o.tile([33, N], f32, tag="av", name="psav")
                        for jt in range(8):
                            for ns in range(2):
                                nc.tensor.matmul(
                                    pso[:, ns * 512:(ns + 1) * 512],
                                    fr(vt_sb[jt][:, 33 * h: 33 * h + 33]),
                                    fr(ets[hi][jt][:, ns * 512:(ns + 1) * 512]),
                                    start=(jt == 0), stop=(jt == 7),
                                )
                        # evict: rows 0:32 = out', row 32 = softmax row-sum
                        zt = ztmpp.tile([33, N], f32, tag="ztmp", name="ztmp")
                        nc.vector.tensor_copy(zt, pso)
                        nc.sync.dma_start(out=r_sb[h: h + 1, :], in_=zt[32:33, :])
                        # compact into z (canonical channel order c = 32h + d)
                        nc.sync.dma_start(
                            out=z_sb[h // 4][32 * (h % 4): 32 * (h % 4) + 32, :],
                            in_=zt[0:32, :],
                        )

                # ---- softmax normalization ----
                rinv_sb = rp.tile([8, N], f32, tag="rinv", name="rinv")
                nc.vector.reciprocal_approx_fast(out=rinv_sb, in_=r_sb)
                for tt in range(2):
                    ps = ps_sm.tile([128, N], f32, tag="mm", name="psmm")
                    for ns in range(2):
                        nc.tensor.matmul(
                            ps[:, ns * 512:(ns + 1) * 512],
                            fr(sel_sb[tt]),
                            fr(rinv_sb[:, ns * 512:(ns + 1) * 512]),
                            start=True, stop=True,
                        )
                    nc.vector.tensor_tensor(z_sb[tt], z_sb[tt], ps, ALU.mult)

                # ---- depthwise 3x3 conv accumulated into z ----
                for tt in range(2):
                    zv = z_sb[tt].rearrange("p (h w) -> p h w", h=HH)
                    vv = vcm_sb[tt].rearrange("p (h w) -> p h w", h=HH)
                    # center tap on the flat view: adds BN bias once
                    nc.vector.affine_then_add(
                        out=z_sb[tt], in0=vcm_sb[tt], in1=z_sb[tt],
                        scale=w9_sb[tt][:, 4:5], bias=bpe_sb[tt],
                    )
                    for tap in range(9):
                        if tap == 4:
                            continue
                        dy, dx = tap // 3 - 1, tap % 3 - 1
                        oh0, oh1 = max(0, -dy), HH - max(0, dy)
                        ow0, ow1 = max(0, -dx), WW - max(0, dx)
                        nc.vector.scalar_tensor_tensor(
                            out=zv[:, oh0:oh1, ow0:ow1],
                            in0=vv[:, oh0 + dy:oh1 + dy, ow0 + dx:ow1 + dx],
                            scalar=w9_sb[tt][:, tap:tap + 1],
                            in1=zv[:, oh0:oh1, ow0:ow1],
                            op0=ALU.mult, op1=ALU.add,
                        )

                # ---- proj + BN + store ----
                for mt in range(2):
                    ps = ps_sm.tile([128, N], f32, tag="mm", name="psmm")
                    for kt in range(2):
                        for ns in range(2):
                            nc.tensor.matmul(
                                ps[:, ns * 512:(ns + 1) * 512],
                                fr(wpj_sb[kt][:, mt * 128:(mt + 1) * 128]),
                                fr(z_sb[kt][:, ns * 512:(ns + 1) * 512]),
                                start=(kt == 0), stop=(kt == 1),
                            )
                    yt = yp.tile([128, N], f32, tag=f"y{mt}", name=f"y{mt}")
                    nc.vector.tensor_scalar_add(yt, ps, bpj_sb[mt])
                    nc.sync.dma_start(out=y_d[b, mt * 128:(mt + 1) * 128, :], in_=yt)

    nc.compile()
    return nc


def _prep_weights(qkv_w, qkv_g, qkv_b, qkv_m, qkv_v,
                  pe_w, pe_g, pe_b, pe_m, pe_v,
                  proj_w, proj_g, proj_b, proj_m, proj_v):
    f = np.float32
    s1 = (qkv_g / np.sqrt(qkv_v + EPS)).astype(f)
    t1 = (qkv_b - qkv_m * s1).astype(f)
    w1 = (qkv_w * s1[:, None]).astype(f)  # [512, 256]

    # q/k padded strip layout: 4 m-tiles of 128 cols.
    wqk = np.zeros((C, 512), f)
    bqk = np.zeros((4, 128, 1), f)
    for h in range(NH):
        g, m = divmod(h, 4)  # g: 0 = heads 0-3, 1 = heads 4-7
        qrows = slice(64 * h, 64 * h + 16)
        krows = slice(64 * h + 16, 64 * h + 32)
        wqk[:, 256 * g + 32 * m: 256 * g + 32 * m + 16] = w1[qrows].T
        wqk[:, 256 * g + 128 + 32 * m: 256 * g + 128 + 32 * m + 16] = w1[krows].T
        bqk[2 * g, 32 * m: 32 * m + 16, 0] = t1[qrows]
        bqk[2 * g + 1, 32 * m: 32 * m + 16, 0] = t1[krows]

    # vT_aug weights [257, 264]
    wva = np.zeros((C + 1, 264), f)
    for h in range(NH):
        vrows = slice(64 * h + 32, 64 * h + 64)
        wva[:C, 33 * h: 33 * h + 32] = w1[vrows].T
        wva[C, 33 * h: 33 * h + 32] = t1[vrows]
        wva[C, 33 * h + 32] = 1.0
    wv = wva[:C].reshape(2, 128, 264)
    wvl = np.repeat(wva[C:C + 1], 128, axis=0)  # bias row replicated

    # v channel-major weights (canonical order c = 32h + d)
    wvcm = np.zeros((C, C), f)
    bvcm = np.zeros((C,), f)
    for h in range(NH):
        vrows = slice(64 * h + 32, 64 * h + 64)
        wvcm[:, 32 * h: 32 * h + 32] = w1[vrows].T
        bvcm[32 * h: 32 * h + 32] = t1[vrows]
    wvcm = wvcm.reshape(2, 128, 256)
    bvcm = bvcm.reshape(2, 128, 1)

    # depthwise conv, pe BN folded
    s2 = (pe_g / np.sqrt(pe_v + EPS)).astype(f)
    t2 = (pe_b - pe_m * s2).astype(f)
    w9 = (pe_w[:, 0].reshape(C, 9) * s2[:, None]).astype(f).reshape(2, 128, 9)
    bpe = t2.reshape(2, 128, 1)

    # proj, BN folded, transposed
    s3 = (proj_g / np.sqrt(proj_v + EPS)).astype(f)
    t3 = (proj_b - proj_m * s3).astype(f)
    w3 = (proj_w * s3[:, None]).astype(f)
    wpj = np.ascontiguousarray(w3.T).reshape(2, 128, 256)
    bpj = t3.reshape(2, 128, 1)

    # selector for rinv broadcast: z tile t row m -> head 4t + m//32
    sel = np.zeros((2, 8, 128), f)
    for tt in range(2):
        for m in range(128):
            sel[tt, 4 * tt + m // 32, m] = 1.0

    return dict(wqk=wqk.reshape(2, 128, 512), bqk=bqk, wv=wv, wvl=wvl,
                wvcm=wvcm, bvcm=bvcm, w9=w9, bpe=bpe, wpj=wpj, bpj=bpj, sel=sel)


def _round_f32r(a):
    """Round fp32 to the PE's fp32r format (11 explicit mantissa bits)."""
    b = np.ascontiguousarray(a, np.float32).view(np.uint32)
    return ((b + np.uint32(0x800)) & np.uint32(0xFFFFF000)).view(np.float32)


def kernel(**inputs):
    from concourse.bass_utils import run_bass_kernel_spmd

    x = np.asarray(inputs["x"], dtype=np.float32)  # [16, 256, 32, 32]
    w = _prep_weights(
        np.asarray(inputs["qkv_w"], np.float32), np.asarray(inputs["qkv_g"], np.float32),
        np.asarray(inputs["qkv_b"], np.float32), np.asarray(inputs["qkv_m"], np.float32),
        np.asarray(inputs["qkv_v"], np.float32),
        np.asarray(inputs["pe_w"], np.float32), np.asarray(inputs["pe_g"], np.float32),
        np.asarray(inputs["pe_b"], np.float32), np.asarray(inputs["pe_m"], np.float32),
        np.asarray(inputs["pe_v"], np.float32),
        np.asarray(inputs["proj_w"], np.float32), np.asarray(inputs["proj_g"], np.float32),
        np.asarray(inputs["proj_b"], np.float32), np.asarray(inputs["proj_m"], np.float32),
        np.asarray(inputs["proj_v"], np.float32),
    )

    if "nc" not in _CACHE:
        _CACHE["nc"] = _build_program()
    nc = _CACHE["nc"]

    if USE_F32R:
        for k in ("wqk", "wv", "wvcm"):
            w[k] = _round_f32r(w[k])
        x = _round_f32r(x)
    xr = x.reshape(B, C, N)
    in_maps = []
    for c in range(NCORES):
        m = {"x": np.ascontiguousarray(xr[c * BPC:(c + 1) * BPC])}
        m.update(w)
        in_maps.append(m)

    res = run_bass_kernel_spmd(nc, in_maps, core_ids=list(range(NCORES)))
    y = np.concatenate([res.results[c]["y"] for c in range(NCORES)], axis=0)
    return y.reshape(B, C, HH, WW).astype(np.float32)


# revision 25
# speedup vs baseline: 1.0964x; 1.0033x over previous
"""Trainium2 Bass kernel for nn_Attention (dense transformer block).

Computation (per batch b of 16):
  qkv = BN(qkv_w @ x)            # 1x1 conv + BN, x: [256, 1024]
  per head h (8): q,k: [16,1024], v: [32,1024]
  attn = softmax(q^T k * scale); out = v @ attn^T
  pe   = BN(dwconv3x3(v_img))
  y    = BN(proj_w @ (out + pe))

Sharding: data-parallel over batch. 16 batches / 8 cores = 2 per core.
All BN folds / weight transposes / layout packing happen on host (numpy).

Device-side layout tricks:
 - q/k packed at 32-aligned partition strips so the K=16 logits matmuls
   row-tile into concurrent PE strips.
 - S_T = k^T q computed *transposed* ([j,i]) so softmax's contraction dim
   lands on partitions for the AV matmul -- no transposes needed.
 - v^T computed directly from x via x^T @ Wv with an appended ones column:
   the AV matmul then yields the softmax denominator for free (M=33).
 - attention runs per head-pair: S_T matmuls for the two heads alternate
   32-row PE strips (tile_position row tiling) and their AV matmuls are
   deferred into bf16 E-tiles so they overlap the next pair's S_T/exp.
 - exp on ACT reads [128,1024] PSUM tiles; q/k/E/vT are bf16 (softmax
   normalization cancels most of the rounding), big GEMMs are fp32r.
 - softmax normalization: row-sums DMA'd to a [8,1024] tile, reciprocal via
   1 custom-DVE op, broadcast back with a K=8 selector matmul.
 - depthwise conv: 9 scalar_tensor_tensor DVE ops (shifted APs, zero pad
   via clipped slices) accumulating into the attention output.
PSUM budget (8 banks): S_T producer 2x[128,1024] (4) + AV accumulator
[33,1024] (2) + stage-1/proj/bcast pool [128,1024] (2).
"""

import numpy as np

B, C, HH, WW = 16, 256, 32, 32
N = HH * WW  # 1024
NH, HD, KD = 8, 32, 16
SCALE = KD ** -0.5
QKV_CH = C + 2 * KD * NH  # 512
EPS = 1e-5
NCORES = 8
BPC = B // NCORES  # batches per core

# matmul dtype: float32r = fp32 bits, PE reduced-precision full-rate mode.
USE_F32R = True

_CACHE = {}


def _build_program():
    import concourse.bass as bass
    import concourse.bacc as bacc
    import concourse.tile as tile
    from concourse import mybir

    f32 = mybir.dt.float32
    f32r = mybir.dt.float32r
    AF = mybir.ActivationFunctionType
    ALU = mybir.AluOpType

    mdt = f32r if USE_F32R else f32
    bdt = mybir.dt.bfloat16

    def fr(ap):
        return ap

    nc = bacc.Bacc("TRN2", target_bir_lowering=False, debug=False)

    # ---- DRAM I/O ----
    x_d = nc.dram_tensor("x", [BPC, C, N], mdt, kind="ExternalInput").ap()
    wqk_d = nc.dram_tensor("wqk", [2, 128, 512], mdt, kind="ExternalInput").ap()
    bqk_d = nc.dram_tensor("bqk", [4, 128, 1], f32, kind="ExternalInput").ap()
    wv_d = nc.dram_tensor("wv", [2, 128, 264], mdt, kind="ExternalInput").ap()
    wvl_d = nc.dram_tensor("wvl", [128, 264], f32, kind="ExternalInput").ap()
    wvcm_d = nc.dram_tensor("wvcm", [2, 128, 256], mdt, kind="ExternalInput").ap()
    bvcm_d = nc.dram_tensor("bvcm", [2, 128, 1], f32, kind="ExternalInput").ap()
    w9_d = nc.dram_tensor("w9", [2, 128, 9], f32, kind="ExternalInput").ap()
    bpe_d = nc.dram_tensor("bpe", [2, 128, 1], f32, kind="ExternalInput").ap()
    wpj_d = nc.dram_tensor("wpj", [2, 128, 256], f32, kind="ExternalInput").ap()
    bpj_d = nc.dram_tensor("bpj", [2, 128, 1], f32, kind="ExternalInput").ap()
    sel_d = nc.dram_tensor("sel", [2, 8, 128], f32, kind="ExternalInput").ap()
    y_d = nc.dram_tensor("y", [BPC, C, N], f32, kind="ExternalOutput").ap()

    with tile.TileContext(nc) as tc:
        with (
            tc.tile_pool(name="consts", bufs=1) as consts,
            tc.tile_pool(name="xin", bufs=2) as xin,
            tc.tile_pool(name="qk", bufs=2) as qkp,
            tc.tile_pool(name="vt", bufs=2) as vtp,
            tc.tile_pool(name="vcm", bufs=2) as vcmp,
            tc.tile_pool(name="et", bufs=64) as etp,
            tc.tile_pool(name="ztmp", bufs=2) as ztmpp,
            tc.tile_pool(name="zz", bufs=2) as zp,
            tc.tile_pool(name="rr", bufs=2) as rp,
            tc.tile_pool(name="yy", bufs=2) as yp,
            tc.tile_pool(name="ps_s", bufs=4, space="PSUM") as ps_s,
            tc.tile_pool(name="ps_o", bufs=1, space="PSUM") as ps_o,
            tc.tile_pool(name="ps_sm", bufs=1, space="PSUM") as ps_sm,
        ):
            # ---- load constants ----
            wqk_sb = []
            for kt in range(2):
                t = consts.tile([128, 512], mdt, tag=f"wqk{kt}", name=f"wqk{kt}")
                nc.sync.dma_start(out=t, in_=wqk_d[kt])
                wqk_sb.append(t)
            bqk_sb = []
            for mt in range(4):
                t = consts.tile([128, 1], f32, tag=f"bqk{mt}", name=f"bqk{mt}")
                nc.sync.dma_start(out=t, in_=bqk_d[mt])
                bqk_sb.append(t)
            wv_sb = []
            for kt in range(2):
                t = consts.tile([128, 264], mdt, tag=f"wv{kt}", name=f"wv{kt}")
                nc.sync.dma_start(out=t, in_=wv_d[kt])
                wv_sb.append(t)
            wvl_sb = consts.tile([128, 264], f32, tag="wvl", name="wvl")
            nc.sync.dma_start(out=wvl_sb, in_=wvl_d)
            wvcm_sb = []
            bvcm_sb = []
            w9_sb = []
            bpe_sb = []
            wpj_sb = []
            bpj_sb = []
            for kt in range(2):
                t = consts.tile([128, 256], mdt, tag=f"wvcm{kt}", name=f"wvcm{kt}")
                nc.sync.dma_start(out=t, in_=wvcm_d[kt])
                wvcm_sb.append(t)
                t = consts.tile([128, 1], f32, tag=f"bvcm{kt}", name=f"bvcm{kt}")
                nc.sync.dma_start(out=t, in_=bvcm_d[kt])
                bvcm_sb.append(t)
                t = consts.tile([128, 9], f32, tag=f"w9{kt}", name=f"w9{kt}")
                nc.sync.dma_start(out=t, in_=w9_d[kt])
                w9_sb.append(t)
                t = consts.tile([128, 1], f32, tag=f"bpe{kt}", name=f"bpe{kt}")
                nc.sync.dma_start(out=t, in_=bpe_d[kt])
                bpe_sb.append(t)
                t = consts.tile([128, 256], f32, tag=f"wpj{kt}", name=f"wpj{kt}")
                nc.sync.dma_start(out=t, in_=wpj_d[kt])
                wpj_sb.append(t)
                t = consts.tile([128, 1], f32, tag=f"bpj{kt}", name=f"bpj{kt}")
                nc.sync.dma_start(out=t, in_=bpj_d[kt])
                bpj_sb.append(t)
            sel_sb = []
            for tt in range(2):
                t = consts.tile([8, 128], f32, tag=f"sel{tt}", name=f"sel{tt}")
                nc.sync.dma_start(out=t, in_=sel_d[tt])
                sel_sb.append(t)

            for b in range(BPC):
                # ---- load x ----
                x_sb = []
                for kt in range(2):
                    t = xin.tile([128, N], mdt, tag=f"x{kt}", name=f"x{kt}")
                    nc.sync.dma_start(out=t, in_=x_d[b, kt * 128:(kt + 1) * 128, :])
                    x_sb.append(t)

                # ---- stage 1: q/k rows (padded strip layout) ----
                # m-tile 0: QA (q of heads 0-3), 1: KA, 2: QB (heads 4-7), 3: KB
                qk_sb = []
                for mt in range(4):
                    ps = ps_sm.tile([128, N], f32, tag="mm", name="psmm")
                    for kt in range(2):
                        for ns in range(2):
                            nc.tensor.matmul(
                                ps[:, ns * 512:(ns + 1) * 512],
                                fr(wqk_sb[kt][:, mt * 128:(mt + 1) * 128]),
                                fr(x_sb[kt][:, ns * 512:(ns + 1) * 512]),
                                start=(kt == 0), stop=(kt == 1),
                            )
                    t = qkp.tile([128, N], bdt, tag=f"qk{mt}", name=f"qk{mt}")
                    nc.vector.tensor_scalar_add(t, ps, bqk_sb[mt])
                    qk_sb.append(t)

                # ---- stage 1b: vT_aug = xa^T @ Wv  ([1024, 264]) ----
                vt_sb = []
                for nt in range(8):
                    ps = ps_sm.tile([128, 264], f32, tag="mm", name="psvt")
                    nc.tensor.matmul(
                        ps, fr(x_sb[0][:, nt * 128:(nt + 1) * 128]), fr(wv_sb[0]),
                        start=True, stop=False)
                    nc.tensor.matmul(
                        ps, fr(x_sb[1][:, nt * 128:(nt + 1) * 128]), fr(wv_sb[1]),
                        start=False, stop=True)
                    t = vtp.tile([128, 264], bdt, tag=f"vt{nt}", name=f"vt{nt}")
                    nc.vector.tensor_add(t, ps, wvl_sb)
                    vt_sb.append(t)

                # ---- stage 1c: v channel-major [256, 1024] ----
                vcm_sb = []
                for mt in range(2):
                    ps = ps_sm.tile([128, N], f32, tag="mm", name="psmm")
                    for kt in range(2):
                        for ns in range(2):
                            nc.tensor.matmul(
                                ps[:, ns * 512:(ns + 1) * 512],
                                fr(wvcm_sb[kt][:, mt * 128:(mt + 1) * 128]),
                                fr(x_sb[kt][:, ns * 512:(ns + 1) * 512]),
                                start=(kt == 0), stop=(kt == 1),
                            )
                    t = vcmp.tile([128, N], f32, tag=f"vcm{mt}", name=f"vcm{mt}")
                    nc.vector.tensor_scalar_add(t, ps, bvcm_sb[mt])
                    vcm_sb.append(t)

                # ---- attention: head quads; 4-strip-rotating S_T; deferred AV ----
                z_sb = [zp.tile([128, N], f32, tag=f"z{t}", name=f"z{t}") for t in range(2)]
                r_sb = rp.tile([8, N], f32, tag="r", name="rsum")
                for q in range(2):
                    heads = [4 * q + i for i in range(4)]
                    qt = qk_sb[0 if q == 0 else 2]
                    kt_ = qk_sb[1 if q == 0 else 3]
                    ets = [[None] * 16 for _ in range(4)]  # per head: jt*2+ns
                    # phase A: S_T rotating all 4 strips + exp per half-tile
                    for jt in range(8):
                        for ns in range(2):
                            psss = []
                            for hi in range(4):
                                pss = ps_s.tile([128, 512], f32, tag="st", name="psst")
                                psss.append(pss)
                            for hi in range(4):
                                s = 32 * hi
                                nc.tensor.matmul(
                                    psss[hi],
                                    fr(kt_[s:s + 16, jt * 128:(jt + 1) * 128]),
                                    fr(qt[s:s + 16, ns * 512:(ns + 1) * 512]),
                                    start=True, stop=True,
                                    tile_position=(s, 0),
                                )
                            for hi in range(4):
                                et = etp.tile([128, 512], bdt, tag="et", name="et")
                                nc.scalar.activation(et, psss[hi], AF.Exp, scale=SCALE)
                                ets[hi][2 * jt + ns] = et
                    # phase B: AV per head (overlaps next quad's phase A on PE)
                    for hi, h in enumerate(heads):
                        pso = ps_o.tile([33, N], f32, tag="av", name="psav")
                        for jt in range(8):
                            for ns in range(2):
                                nc.tensor.matmul(
                                    pso[:, ns * 512:(ns + 1) * 512],
                                    fr(vt_sb[jt][:, 33 * h: 33 * h + 33]),
                                    fr(ets[hi][2 * jt + ns]),
                                    start=(jt == 0), stop=(jt == 7),
                                )
                        # evict: rows 0:32 = out', row 32 = softmax row-sum
                        zt = ztmpp.tile([33, N], f32, tag="ztmp", name="ztmp")
                        nc.vector.tensor_copy(zt, pso)
                        nc.sync.dma_start(out=r_sb[h: h + 1, :], in_=zt[32:33, :])
                        # compact into z (canonical channel order c = 32h + d)
                        nc.sync.dma_start(
                            out=z_sb[h // 4][32 * (h % 4): 32 * (h % 4) + 32, :],
                            in_=zt[0:32, :],
                        )

                # ---- softmax normalization ----
                rinv_sb = rp.tile([8, N], f32, tag="rinv", name="rinv")
                nc.vector.reciprocal_approx_fast(out=rinv_sb, in_=r_sb)
                for tt in range(2):
                    ps = ps_sm.tile([128, N], f32, tag="mm", name="psmm")
                    for ns in range(2):
                        nc.tensor.matmul(
                            ps[:, ns * 512:(ns + 1) * 512],
                            fr(sel_sb[tt]),
                            fr(rinv_sb[:, ns * 512:(ns + 1) * 512]),
                            start=True, stop=True,
                        )
                    nc.vector.tensor_tensor(z_sb[tt], z_sb[tt], ps, ALU.mult)

                # ---- depthwise 3x3 conv accumulated into z ----
                for tt in range(2):
                    zv = z_sb[tt].rearrange("p (h w) -> p h w", h=HH)
                    vv = vcm_sb[tt].rearrange("p (h w) -> p h w", h=HH)
                    # center tap on the flat view: adds BN bias once
                    nc.vector.affine_then_add(
                        out=z_sb[tt], in0=vcm_sb[tt], in1=z_sb[tt],
                        scale=w9_sb[tt][:, 4:5], bias=bpe_sb[tt],
                    )
                    for tap in range(9):
                        if tap == 4:
                            continue
                        dy, dx = tap // 3 - 1, tap % 3 - 1
                        oh0, oh1 = max(0, -dy), HH - max(0, dy)
                        ow0, ow1 = max(0, -dx), WW - max(0, dx)
                        nc.vector.scalar_tensor_tensor(
                            out=zv[:, oh0:oh1, ow0:ow1],
                            in0=vv[:, oh0 + dy:oh1 + dy, ow0 + dx:ow1 + dx],
                            scalar=w9_sb[tt][:, tap:tap + 1],
                            in1=zv[:, oh0:oh1, ow0:ow1],
                            op0=ALU.mult, op1=ALU.add,
                        )

                # ---- proj + BN + store ----
                for mt in range(2):
                    ps = ps_sm.tile([128, N], f32, tag="mm", name="psmm")
                    for kt in range(2):
                        for ns in range(2):
                            nc.tensor.matmul(
                                ps[:, ns * 512:(ns + 1) * 512],
                                fr(wpj_sb[kt][:, mt * 128:(mt + 1) * 128]),
                                fr(z_sb[kt][:, ns * 512:(ns + 1) * 512]),
                                start=(kt == 0), stop=(kt == 1),
                            )
                    yt = yp.tile([128, N], f32, tag=f"y{mt}", name=f"y{mt}")
                    nc.vector.tensor_scalar_add(yt, ps, bpj_sb[mt])
                    nc.sync.dma_start(out=y_d[b, mt * 128:(mt + 1) * 128, :], in_=yt)

    nc.compile()
    return nc


def _prep_weights(qkv_w, qkv_g, qkv_b, qkv_m, qkv_v,
                  pe_w, pe_g, pe_b, pe_m, pe_v,
                  proj_w, proj_g, proj_b, proj_m, proj_v):
    f = np.float32
    s1 = (qkv_g / np.sqrt(qkv_v + EPS)).astype(f)
    t1 = (qkv_b - qkv_m * s1).astype(f)
    w1 = (qkv_w * s1[:, None]).astype(f)  # [512, 256]

    # q/k padded strip layout: 4 m-tiles of 128 cols.
    wqk = np.zeros((C, 512), f)
    bqk = np.zeros((4, 128, 1), f)
    for h in range(NH):
        g, m = divmod(h, 4)  # g: 0 = heads 0-3, 1 = heads 4-7
        qrows = slice(64 * h, 64 * h + 16)
        krows = slice(64 * h + 16, 64 * h + 32)
        wqk[:, 256 * g + 32 * m: 256 * g + 32 * m + 16] = w1[qrows].T
        wqk[:, 256 * g + 128 + 32 * m: 256 * g + 128 + 32 * m + 16] = w1[krows].T
        bqk[2 * g, 32 * m: 32 * m + 16, 0] = t1[qrows]
        bqk[2 * g + 1, 32 * m: 32 * m + 16, 0] = t1[krows]

    # vT_aug weights [257, 264]
    wva = np.zeros((C + 1, 264), f)
    for h in range(NH):
        vrows = slice(64 * h + 32, 64 * h + 64)
        wva[:C, 33 * h: 33 * h + 32] = w1[vrows].T
        wva[C, 33 * h: 33 * h + 32] = t1[vrows]
        wva[C, 33 * h + 32] = 1.0
    wv = wva[:C].reshape(2, 128, 264)
    wvl = np.repeat(wva[C:C + 1], 128, axis=0)  # bias row replicated

    # v channel-major weights (canonical order c = 32h + d)
    wvcm = np.zeros((C, C), f)
    bvcm = np.zeros((C,), f)
    for h in range(NH):
        vrows = slice(64 * h + 32, 64 * h + 64)
        wvcm[:, 32 * h: 32 * h + 32] = w1[vrows].T
        bvcm[32 * h: 32 * h + 32] = t1[vrows]
    wvcm = wvcm.reshape(2, 128, 256)
    bvcm = bvcm.reshape(2, 128, 1)

    # depthwise conv, pe BN folded
    s2 = (pe_g / np.sqrt(pe_v + EPS)).astype(f)
    t2 = (pe_b - pe_m * s2).astype(f)
    w9 = (pe_w[:, 0].reshape(C, 9) * s2[:, None]).astype(f).reshape(2, 128, 9)
    bpe = t2.reshape(2, 128, 1)

    # proj, BN folded, transposed
    s3 = (proj_g / np.sqrt(proj_v + EPS)).astype(f)
    t3 = (proj_b - proj_m * s3).astype(f)
    w3 = (proj_w * s3[:, None]).astype(f)
    wpj = np.ascontiguousarray(w3.T).reshape(2, 128, 256)
    bpj = t3.reshape(2, 128, 1)

    # selector for rinv broadcast: z tile t row m -> head 4t + m//32
    sel = np.zeros((2, 8, 128), f)
    for tt in range(2):
        for m in range(128):
            sel[tt, 4 * tt + m // 32, m] = 1.0

    return dict(wqk=wqk.reshape(2, 128, 512), bqk=bqk, wv=wv, wvl=wvl,
                wvcm=wvcm, bvcm=bvcm, w9=w9, bpe=bpe, wpj=wpj, bpj=bpj, sel=sel)


def _round_f32r(a):
    """Round fp32 to the PE's fp32r format (11 explicit mantissa bits)."""
    b = np.ascontiguousarray(a, np.float32).view(np.uint32)
    return ((b + np.uint32(0x800)) & np.uint32(0xFFFFF000)).view(np.float32)


def kernel(**inputs):
    from concourse.bass_utils import run_bass_kernel_spmd

    x = np.asarray(inputs["x"], dtype=np.float32)  # [16, 256, 32, 32]
    w = _prep_weights(
        np.asarray(inputs["qkv_w"], np.float32), np.asarray(inputs["qkv_g"], np.float32),
        np.asarray(inputs["qkv_b"], np.float32), np.asarray(inputs["qkv_m"], np.float32),
        np.asarray(inputs["qkv_v"], np.float32),
        np.asarray(inputs["pe_w"], np.float32), np.asarray(inputs["pe_g"], np.float32),
        np.asarray(inputs["pe_b"], np.float32), np.asarray(inputs["pe_m"], np.float32),
        np.asarray(inputs["pe_v"], np.float32),
        np.asarray(inputs["proj_w"], np.float32), np.asarray(inputs["proj_g"], np.float32),
        np.asarray(inputs["proj_b"], np.float32), np.asarray(inputs["proj_m"], np.float32),
        np.asarray(inputs["proj_v"], np.float32),
    )

    if "nc" not in _CACHE:
        _CACHE["nc"] = _build_program()
    nc = _CACHE["nc"]

    if USE_F32R:
        for k in ("wqk", "wv", "wvcm"):
            w[k] = _round_f32r(w[k])
        x = _round_f32r(x)
    xr = x.reshape(B, C, N)
    in_maps = []
    for c in range(NCORES):
        m = {"x": np.ascontiguousarray(xr[c * BPC:(c + 1) * BPC])}
        m.update(w)
        in_maps.append(m)

    res = run_bass_kernel_spmd(nc, in_maps, core_ids=list(range(NCORES)))
    y = np.concatenate([res.results[c]["y"] for c in range(NCORES)], axis=0)
    return y.reshape(B, C, HH, WW).astype(np.float32)


# revision 26
# speedup vs baseline: 1.1191x; 1.0207x over previous
"""Trainium2 Bass kernel for nn_Attention (dense transformer block).

Computation (per batch b of 16):
  qkv = BN(qkv_w @ x)            # 1x1 conv + BN, x: [256, 1024]
  per head h (8): q,k: [16,1024], v: [32,1024]
  attn = softmax(q^T k * scale); out = v @ attn^T
  pe   = BN(dwconv3x3(v_img))
  y    = BN(proj_w @ (out + pe))

Sharding: data-parallel over batch. 16 batches / 8 cores = 2 per core.
All BN folds / weight transposes / layout packing happen on host (numpy).

Device-side layout tricks:
 - q/k packed at 32-aligned partition strips so the K=16 logits matmuls
   row-tile into concurrent PE strips.
 - S_T = k^T q computed *transposed* ([j,i]) so softmax's contraction dim
   lands on partitions for the AV matmul -- no transposes needed.
 - v^T computed directly from x via x^T @ Wv with an appended ones column:
   the AV matmul then yields the softmax denominator for free (M=33).
 - attention runs per head-pair: S_T matmuls for the two heads alternate
   32-row PE strips (tile_position row tiling) and their AV matmuls are
   deferred into bf16 E-tiles so they overlap the next pair's S_T/exp.
 - exp on ACT reads [128,1024] PSUM tiles; q/k/E/vT are bf16 (softmax
   normalization cancels most of the rounding), big GEMMs are fp32r.
 - softmax normalization: row-sums DMA'd to a [8,1024] tile, reciprocal via
   1 custom-DVE op, broadcast back with a K=8 selector matmul.
 - depthwise conv: 9 scalar_tensor_tensor DVE ops (shifted APs, zero pad
   via clipped slices) accumulating into the attention output.
PSUM budget (8 banks): S_T producer 2x[128,1024] (4) + AV accumulator
[33,1024] (2) + stage-1/proj/bcast pool [128,1024] (2).
"""

import numpy as np

B, C, HH, WW = 16, 256, 32, 32
N = HH * WW  # 1024
NH, HD, KD = 8, 32, 16
SCALE = KD ** -0.5
QKV_CH = C + 2 * KD * NH  # 512
EPS = 1e-5
NCORES = 8
BPC = B // NCORES  # batches per core

# matmul dtype: float32r = fp32 bits, PE reduced-precision full-rate mode.
USE_F32R = True

_CACHE = {}


def _build_program():
    import concourse.bass as bass
    import concourse.bacc as bacc
    import concourse.tile as tile
    from concourse import mybir

    f32 = mybir.dt.float32
    f32r = mybir.dt.float32r
    AF = mybir.ActivationFunctionType
    ALU = mybir.AluOpType

    mdt = f32r if USE_F32R else f32
    bdt = mybir.dt.bfloat16

    def fr(ap):
        return ap

    nc = bacc.Bacc("TRN2", target_bir_lowering=False, debug=False)

    # ---- DRAM I/O ----
    x_d = nc.dram_tensor("x", [BPC, C, N], mdt, kind="ExternalInput").ap()
    wqk_d = nc.dram_tensor("wqk", [2, 128, 512], mdt, kind="ExternalInput").ap()
    bqk_d = nc.dram_tensor("bqk", [4, 128, 1], f32, kind="ExternalInput").ap()
    wv_d = nc.dram_tensor("wv", [2, 128, 264], mdt, kind="ExternalInput").ap()
    wvl_d = nc.dram_tensor("wvl", [128, 264], f32, kind="ExternalInput").ap()
    wvcm_d = nc.dram_tensor("wvcm", [2, 128, 256], mdt, kind="ExternalInput").ap()
    bvcm_d = nc.dram_tensor("bvcm", [2, 128, 1], f32, kind="ExternalInput").ap()
    w9_d = nc.dram_tensor("w9", [2, 128, 9], f32, kind="ExternalInput").ap()
    bpe_d = nc.dram_tensor("bpe", [2, 128, 1], f32, kind="ExternalInput").ap()
    wpj_d = nc.dram_tensor("wpj", [2, 128, 256], f32, kind="ExternalInput").ap()
    bpj_d = nc.dram_tensor("bpj", [2, 128, 1], f32, kind="ExternalInput").ap()
    sel_d = nc.dram_tensor("sel", [2, 8, 128], mdt, kind="ExternalInput").ap()
    y_d = nc.dram_tensor("y", [BPC, C, N], f32, kind="ExternalOutput").ap()

    with tile.TileContext(nc) as tc:
        with (
            tc.tile_pool(name="consts", bufs=1) as consts,
            tc.tile_pool(name="xin", bufs=2) as xin,
            tc.tile_pool(name="qk", bufs=2) as qkp,
            tc.tile_pool(name="vt", bufs=2) as vtp,
            tc.tile_pool(name="vcm", bufs=2) as vcmp,
            tc.tile_pool(name="et", bufs=64) as etp,
            tc.tile_pool(name="ztmp", bufs=2) as ztmpp,
            tc.tile_pool(name="zz", bufs=2) as zp,
            tc.tile_pool(name="rr", bufs=2) as rp,
            tc.tile_pool(name="yy", bufs=2) as yp,
            tc.tile_pool(name="ps_s", bufs=4, space="PSUM") as ps_s,
            tc.tile_pool(name="ps_o", bufs=1, space="PSUM") as ps_o,
            tc.tile_pool(name="ps_sm", bufs=1, space="PSUM") as ps_sm,
        ):
            # ---- load constants ----
            wqk_sb = []
            for kt in range(2):
                t = consts.tile([128, 512], mdt, tag=f"wqk{kt}", name=f"wqk{kt}")
                nc.sync.dma_start(out=t, in_=wqk_d[kt])
                wqk_sb.append(t)
            bqk_sb = []
            for mt in range(4):
                t = consts.tile([128, 1], f32, tag=f"bqk{mt}", name=f"bqk{mt}")
                nc.sync.dma_start(out=t, in_=bqk_d[mt])
                bqk_sb.append(t)
            wv_sb = []
            for kt in range(2):
                t = consts.tile([128, 264], mdt, tag=f"wv{kt}", name=f"wv{kt}")
                nc.sync.dma_start(out=t, in_=wv_d[kt])
                wv_sb.append(t)
            wvl_sb = consts.tile([128, 264], f32, tag="wvl", name="wvl")
            nc.sync.dma_start(out=wvl_sb, in_=wvl_d)
            wvcm_sb = []
            bvcm_sb = []
            w9_sb = []
            bpe_sb = []
            wpj_sb = []
            bpj_sb = []
            for kt in range(2):
                t = consts.tile([128, 256], mdt, tag=f"wvcm{kt}", name=f"wvcm{kt}")
                nc.sync.dma_start(out=t, in_=wvcm_d[kt])
                wvcm_sb.append(t)
                t = consts.tile([128, 1], f32, tag=f"bvcm{kt}", name=f"bvcm{kt}")
                nc.sync.dma_start(out=t, in_=bvcm_d[kt])
                bvcm_sb.append(t)
                t = consts.tile([128, 9], f32, tag=f"w9{kt}", name=f"w9{kt}")
                nc.sync.dma_start(out=t, in_=w9_d[kt])
                w9_sb.append(t)
                t = consts.tile([128, 1], f32, tag=f"bpe{kt}", name=f"bpe{kt}")
                nc.sync.dma_start(out=t, in_=bpe_d[kt])
                bpe_sb.append(t)
                t = consts.tile([128, 256], f32, tag=f"wpj{kt}", name=f"wpj{kt}")
                nc.sync.dma_start(out=t, in_=wpj_d[kt])
                wpj_sb.append(t)
                t = consts.tile([128, 1], f32, tag=f"bpj{kt}", name=f"bpj{kt}")
                nc.sync.dma_start(out=t, in_=bpj_d[kt])
                bpj_sb.append(t)
            sel_sb = []
            for tt in range(2):
                t = consts.tile([8, 128], mdt, tag=f"sel{tt}", name=f"sel{tt}")
                nc.sync.dma_start(out=t, in_=sel_d[tt])
                sel_sb.append(t)

            for b in range(BPC):
                # ---- load x ----
                x_sb = []
                for kt in range(2):
                    t = xin.tile([128, N], mdt, tag=f"x{kt}", name=f"x{kt}")
                    nc.sync.dma_start(out=t, in_=x_d[b, kt * 128:(kt + 1) * 128, :])
                    x_sb.append(t)

                # ---- stage 1: q/k rows (padded strip layout) ----
                # m-tile 0: QA (q of heads 0-3), 1: KA, 2: QB (heads 4-7), 3: KB
                qk_sb = []
                for mt in range(4):
                    ps = ps_sm.tile([128, N], f32, tag="mm", name="psmm")
                    for kt in range(2):
                        for ns in range(2):
                            nc.tensor.matmul(
                                ps[:, ns * 512:(ns + 1) * 512],
                                fr(wqk_sb[kt][:, mt * 128:(mt + 1) * 128]),
                                fr(x_sb[kt][:, ns * 512:(ns + 1) * 512]),
                                start=(kt == 0), stop=(kt == 1),
                            )
                    t = qkp.tile([128, N], bdt, tag=f"qk{mt}", name=f"qk{mt}")
                    nc.vector.tensor_scalar_add(t, ps, bqk_sb[mt])
                    qk_sb.append(t)

                # ---- stage 1b: vT_aug = xa^T @ Wv  ([1024, 264]) ----
                vt_sb = []
                for nt in range(8):
                    ps = ps_sm.tile([128, 264], f32, tag="mm", name="psvt")
                    nc.tensor.matmul(
                        ps, fr(x_sb[0][:, nt * 128:(nt + 1) * 128]), fr(wv_sb[0]),
                        start=True, stop=False)
                    nc.tensor.matmul(
                        ps, fr(x_sb[1][:, nt * 128:(nt + 1) * 128]), fr(wv_sb[1]),
                        start=False, stop=True)
                    t = vtp.tile([128, 264], bdt, tag=f"vt{nt}", name=f"vt{nt}")
                    nc.vector.tensor_add(t, ps, wvl_sb)
                    vt_sb.append(t)

                # ---- stage 1c: v channel-major [256, 1024] ----
                vcm_sb = []
                for mt in range(2):
                    ps = ps_sm.tile([128, N], f32, tag="mm", name="psmm")
                    for kt in range(2):
                        for ns in range(2):
                            nc.tensor.matmul(
                                ps[:, ns * 512:(ns + 1) * 512],
                                fr(wvcm_sb[kt][:, mt * 128:(mt + 1) * 128]),
                                fr(x_sb[kt][:, ns * 512:(ns + 1) * 512]),
                                start=(kt == 0), stop=(kt == 1),
                            )
                    t = vcmp.tile([128, N], f32, tag=f"vcm{mt}", name=f"vcm{mt}")
                    nc.vector.tensor_scalar_add(t, ps, bvcm_sb[mt])
                    vcm_sb.append(t)

                # ---- attention: head quads; 4-strip-rotating S_T; deferred AV ----
                z_sb = [zp.tile([128, N], f32, tag=f"z{t}", name=f"z{t}") for t in range(2)]
                r_sb = rp.tile([8, N], f32, tag="r", name="rsum")
                for q in range(2):
                    heads = [4 * q + i for i in range(4)]
                    qt = qk_sb[0 if q == 0 else 2]
                    kt_ = qk_sb[1 if q == 0 else 3]
                    ets = [[None] * 16 for _ in range(4)]  # per head: jt*2+ns
                    # phase A: S_T rotating all 4 strips + exp per half-tile
                    for jt in range(8):
                        for ns in range(2):
                            psss = []
                            for hi in range(4):
                                pss = ps_s.tile([128, 512], f32, tag="st", name="psst")
                                psss.append(pss)
                            for hi in range(4):
                                s = 32 * hi
                                nc.tensor.matmul(
                                    psss[hi],
                                    fr(kt_[s:s + 16, jt * 128:(jt + 1) * 128]),
                                    fr(qt[s:s + 16, ns * 512:(ns + 1) * 512]),
                                    start=True, stop=True,
                                    tile_position=(s, 0),
                                )
                            for hi in range(4):
                                et = etp.tile([128, 512], bdt, tag="et", name="et")
                                nc.scalar.activation(et, psss[hi], AF.Exp, scale=SCALE)
                                ets[hi][2 * jt + ns] = et
                    # phase B: AV per head (overlaps next quad's phase A on PE)
                    for hi, h in enumerate(heads):
                        pso = ps_o.tile([33, N], f32, tag="av", name="psav")
                        for jt in range(8):
                            for ns in range(2):
                                nc.tensor.matmul(
                                    pso[:, ns * 512:(ns + 1) * 512],
                                    fr(vt_sb[jt][:, 33 * h: 33 * h + 33]),
                                    fr(ets[hi][2 * jt + ns]),
                                    start=(jt == 0), stop=(jt == 7),
                                )
                        # evict: rows 0:32 = out', row 32 = softmax row-sum
                        zt = ztmpp.tile([33, N], f32, tag="ztmp", name="ztmp")
                        nc.vector.tensor_copy(zt, pso)
                        nc.sync.dma_start(out=r_sb[h: h + 1, :], in_=zt[32:33, :])
                        # compact into z (canonical channel order c = 32h + d)
                        nc.sync.dma_start(
                            out=z_sb[h // 4][32 * (h % 4): 32 * (h % 4) + 32, :],
                            in_=zt[0:32, :],
                        )

                # ---- softmax normalization ----
                rinv_sb = rp.tile([8, N], f32, tag="rinv", name="rinv")
                nc.vector.reciprocal_approx_fast(out=rinv_sb, in_=r_sb)
                rinv_r = rp.tile([8, N], mdt, tag="rinvr", name="rinvr")
                nc.vector.tensor_copy(rinv_r, rinv_sb)
                for tt in range(2):
                    ps = ps_sm.tile([128, N], f32, tag="mm", name="psmm")
                    for ns in range(2):
                        nc.tensor.matmul(
                            ps[:, ns * 512:(ns + 1) * 512],
                            fr(sel_sb[tt]),
                            fr(rinv_r[:, ns * 512:(ns + 1) * 512]),
                            start=True, stop=True,
                        )
                    nc.vector.tensor_tensor(z_sb[tt], z_sb[tt], ps, ALU.mult)

                # ---- depthwise 3x3 conv accumulated into z ----
                for tt in range(2):
                    zv = z_sb[tt].rearrange("p (h w) -> p h w", h=HH)
                    vv = vcm_sb[tt].rearrange("p (h w) -> p h w", h=HH)
                    # center tap on the flat view: adds BN bias once
                    nc.vector.affine_then_add(
                        out=z_sb[tt], in0=vcm_sb[tt], in1=z_sb[tt],
                        scale=w9_sb[tt][:, 4:5], bias=bpe_sb[tt],
                    )
                    for tap in range(9):
                        if tap == 4:
                            continue
                        dy, dx = tap // 3 - 1, tap % 3 - 1
                        oh0, oh1 = max(0, -dy), HH - max(0, dy)
                        ow0, ow1 = max(0, -dx), WW - max(0, dx)
                        nc.vector.scalar_tensor_tensor(
                            out=zv[:, oh0:oh1, ow0:ow1],
                            in0=vv[:, oh0 + dy:oh1 + dy, ow0 + dx:ow1 + dx],
                            scalar=w9_sb[tt][:, tap:tap + 1],
                            in1=zv[:, oh0:oh1, ow0:ow1],
                            op0=ALU.mult, op1=ALU.add,
                        )

                # ---- proj + BN + store ----
                for mt in range(2):
                    ps = ps_sm.tile([128, N], f32, tag="mm", name="psmm")
                    for kt in range(2):
                        for ns in range(2):
                            nc.tensor.matmul(
                                ps[:, ns * 512:(ns + 1) * 512],
                                fr(wpj_sb[kt][:, mt * 128:(mt + 1) * 128]),
                                fr(z_sb[kt][:, ns * 512:(ns + 1) * 512]),
                                start=(kt == 0), stop=(kt == 1),
                            )
                    yt = yp.tile([128, N], f32, tag=f"y{mt}", name=f"y{mt}")
                    nc.vector.tensor_scalar_add(yt, ps, bpj_sb[mt])
                    nc.sync.dma_start(out=y_d[b, mt * 128:(mt + 1) * 128, :], in_=yt)

    nc.compile()
    return nc


def _prep_weights(qkv_w, qkv_g, qkv_b, qkv_m, qkv_v,
                  pe_w, pe_g, pe_b, pe_m, pe_v,
                  proj_w, proj_g, proj_b, proj_m, proj_v):
    f = np.float32
    s1 = (qkv_g / np.sqrt(qkv_v + EPS)).astype(f)
    t1 = (qkv_b - qkv_m * s1).astype(f)
    w1 = (qkv_w * s1[:, None]).astype(f)  # [512, 256]

    # q/k padded strip layout: 4 m-tiles of 128 cols.
    wqk = np.zeros((C, 512), f)
    bqk = np.zeros((4, 128, 1), f)
    for h in range(NH):
        g, m = divmod(h, 4)  # g: 0 = heads 0-3, 1 = heads 4-7
        qrows = slice(64 * h, 64 * h + 16)
        krows = slice(64 * h + 16, 64 * h + 32)
        wqk[:, 256 * g + 32 * m: 256 * g + 32 * m + 16] = w1[qrows].T
        wqk[:, 256 * g + 128 + 32 * m: 256 * g + 128 + 32 * m + 16] = w1[krows].T
        bqk[2 * g, 32 * m: 32 * m + 16, 0] = t1[qrows]
        bqk[2 * g + 1, 32 * m: 32 * m + 16, 0] = t1[krows]

    # vT_aug weights [257, 264]
    wva = np.zeros((C + 1, 264), f)
    for h in range(NH):
        vrows = slice(64 * h + 32, 64 * h + 64)
        wva[:C, 33 * h: 33 * h + 32] = w1[vrows].T
        wva[C, 33 * h: 33 * h + 32] = t1[vrows]
        wva[C, 33 * h + 32] = 1.0
    wv = wva[:C].reshape(2, 128, 264)
    wvl = np.repeat(wva[C:C + 1], 128, axis=0)  # bias row replicated

    # v channel-major weights (canonical order c = 32h + d)
    wvcm = np.zeros((C, C), f)
    bvcm = np.zeros((C,), f)
    for h in range(NH):
        vrows = slice(64 * h + 32, 64 * h + 64)
        wvcm[:, 32 * h: 32 * h + 32] = w1[vrows].T
        bvcm[32 * h: 32 * h + 32] = t1[vrows]
    wvcm = wvcm.reshape(2, 128, 256)
    bvcm = bvcm.reshape(2, 128, 1)

    # depthwise conv, pe BN folded
    s2 = (pe_g / np.sqrt(pe_v + EPS)).astype(f)
    t2 = (pe_b - pe_m * s2).astype(f)
    w9 = (pe_w[:, 0].reshape(C, 9) * s2[:, None]).astype(f).reshape(2, 128, 9)
    bpe = t2.reshape(2, 128, 1)

    # proj, BN folded, transposed
    s3 = (proj_g / np.sqrt(proj_v + EPS)).astype(f)
    t3 = (proj_b - proj_m * s3).astype(f)
    w3 = (proj_w * s3[:, None]).astype(f)
    wpj = np.ascontiguousarray(w3.T).reshape(2, 128, 256)
    bpj = t3.reshape(2, 128, 1)

    # selector for rinv broadcast: z tile t row m -> head 4t + m//32
    sel = np.zeros((2, 8, 128), f)
    for tt in range(2):
        for m in range(128):
            sel[tt, 4 * tt + m // 32, m] = 1.0

    return dict(wqk=wqk.reshape(2, 128, 512), bqk=bqk, wv=wv, wvl=wvl,
                wvcm=wvcm, bvcm=bvcm, w9=w9, bpe=bpe, wpj=wpj, bpj=bpj, sel=sel)


def _round_f32r(a):
    """Round fp32 to the PE's fp32r format (11 explicit mantissa bits)."""
    b = np.ascontiguousarray(a, np.float32).view(np.uint32)
    return ((b + np.uint32(0x800)) & np.uint32(0xFFFFF000)).view(np.float32)


def kernel(**inputs):
    from concourse.bass_utils import run_bass_kernel_spmd

    x = np.asarray(inputs["x"], dtype=np.float32)  # [16, 256, 32, 32]
    w = _prep_weights(
        np.asarray(inputs["qkv_w"], np.float32), np.asarray(inputs["qkv_g"], np.float32),
        np.asarray(inputs["qkv_b"], np.float32), np.asarray(inputs["qkv_m"], np.float32),
        np.asarray(inputs["qkv_v"], np.float32),
        np.asarray(inputs["pe_w"], np.float32), np.asarray(inputs["pe_g"], np.float32),
        np.asarray(inputs["pe_b"], np.float32), np.asarray(inputs["pe_m"], np.float32),
        np.asarray(inputs["pe_v"], np.float32),
        np.asarray(inputs["proj_w"], np.float32), np.asarray(inputs["proj_g"], np.float32),
        np.asarray(inputs["proj_b"], np.float32), np.asarray(inputs["proj_m"], np.float32),
        np.asarray(inputs["proj_v"], np.float32),
    )

    if "nc" not in _CACHE:
        _CACHE["nc"] = _build_program()
    nc = _CACHE["nc"]

    if USE_F32R:
        for k in ("wqk", "wv", "wvcm"):
            w[k] = _round_f32r(w[k])
        x = _round_f32r(x)
    xr = x.reshape(B, C, N)
    in_maps = []
    for c in range(NCORES):
        m = {"x": np.ascontiguousarray(xr[c * BPC:(c + 1) * BPC])}
        m.update(w)
        in_maps.append(m)

    res = run_bass_kernel_spmd(nc, in_maps, core_ids=list(range(NCORES)))
    y = np.concatenate([res.results[c]["y"] for c in range(NCORES)], axis=0)
    return y.reshape(B, C, HH, WW).astype(np.float32)
